# revision 36
# baseline (speedup 1.0000x reference)
"""Trainium2 Bass kernel for the GRU autoencoder.

Distribution strategy (8 NeuronCores):
  Encode : chain-parallel x batch-parallel. Core j handles GRU chain j//2
           (xf, xb, ef, eb) on batch half j%2 (128 rows), running a uniform
           100-step loop. The 50-step x-chains get 50 leading exact identity
           steps (update-gate pre-activation forced to +BIG => z=1 => h'=h).
           Input-side gates, biases and the pad flag ride the same matmul via
           augmented stationary rows (features + ones-row + flag-row).
  Reshard: AllToAll moves 16-row slices so every core assembles the hidden
           states for its own 32-row decode shard at *fixed* (SPMD-uniform)
           indices: core j decodes global rows [16j:16j+16] u [128+16j:+16].
  Middle : per-shard MLP (em1/em2/eo) + decoder const precompute (bf16).
  Decode : 60 autoregressive steps per core on its 32-row shard.

All weights (the matmul MOVING operands) are bf16: the PE streams 16-bit
moving data at 1 col/cycle vs 2 for fp32r, so this halves GEMM time.
Stationary operands (activations) stay fp32r for precision. Weights are
cast to bf16 on the host, halving DMA and removing on-device cast work.

PSUM budget (8 banks): gA 2 | gB 2x1 | gC 2x1 | tr 2x1.
"""

import sys

sys.path.insert(0, "/opt/trn_rl_repo")

import numpy as np
import ml_dtypes

import concourse.bass as bass
import concourse.mybir as mybir
import concourse.tile as tile
from concourse import bacc
from concourse.masks import make_identity

dt = mybir.dt
AF = mybir.ActivationFunctionType
OP = mybir.AluOpType

B, TX, TY, NX, NY, H, HOR = 256, 50, 100, 64, 64, 512, 60
M1, M2 = 1024, 512
G = 3 * H
NCORE = 8
BE = 128   # encoder batch rows per core
BD = 32    # decoder batch rows per core
BIG = 30000.0

F32, F32R, BF16 = dt.float32, dt.float32r, dt.bfloat16
BF16NP = ml_dtypes.bfloat16


def build_nc(et=100, x_real=50, hor=60):
    nc = bacc.Bacc("TRN2", target_bir_lowering=False, debug=False,
                   num_devices=NCORE)

    # ---- DRAM parameters (identical names on every core; content differs) --
    d_xin = nc.dram_tensor("xin", [66, et * BE], BF16, kind="ExternalInput")
    d_wih = nc.dram_tensor("wih_aug", [66, G], BF16, kind="ExternalInput")
    d_whh = nc.dram_tensor("whh_t", [H, G], BF16, kind="ExternalInput")

    d_em1 = nc.dram_tensor("em_w1t", [2 * H, M1], BF16, kind="ExternalInput")
    d_em2 = nc.dram_tensor("em_w2t", [M1, M2], BF16, kind="ExternalInput")
    d_eow = nc.dram_tensor("eo_wt", [M2, H], BF16, kind="ExternalInput")
    d_dcw = nc.dram_tensor("dc_wt", [2 * H, G], BF16, kind="ExternalInput")
    # column-packed bf16 bias rows: dcb|em1b|em2b|eob (3584)
    d_biasb = nc.dram_tensor("bias_b", [1, 3584], BF16, kind="ExternalInput")

    d_dwy = nc.dram_tensor("dwy_t", [NY, G], BF16, kind="ExternalInput")
    d_dwhh = nc.dram_tensor("dwhh_t", [H, G], BF16, kind="ExternalInput")
    d_dm1 = nc.dram_tensor("dm_w1t", [H, M1], BF16, kind="ExternalInput")
    d_dm2 = nc.dram_tensor("dm_w2t", [M1, M2], BF16, kind="ExternalInput")
    d_dow = nc.dram_tensor("do_wt", [M2, NY], BF16, kind="ExternalInput")
    # column-packed bf16 bias rows: bhhn|dbhhn|dm1b|dm2b|dob (2624)
    d_biasr = nc.dram_tensor("bias_r", [1, 2624], BF16, kind="ExternalInput")
    d_xlast = nc.dram_tensor("xlast_t", [NX, BD], BF16, kind="ExternalInput")

    d_out = nc.dram_tensor("out", [BD, hor * NY], F32, kind="ExternalOutput")

    cc_in = nc.dram_tensor("cc_in", [BE, H], F32)
    cc_out = nc.dram_tensor("cc_out", [NCORE, 16, H], F32)

    with tile.TileContext(nc) as tc:
        with tc.tile_pool(name="wenc", bufs=1) as wenc, \
             tc.tile_pool(name="wdec", bufs=1) as wdec, \
             tc.tile_pool(name="state", bufs=2) as st, \
             tc.tile_pool(name="xsp", bufs=4) as xsp, \
             tc.tile_pool(name="tmp", bufs=2) as tp, \
             tc.tile_pool(name="tmp1", bufs=1) as tq, \
             tc.tile_pool(name="mid", bufs=1) as md, \
             tc.tile_pool(name="persist", bufs=1) as pe, \
             tc.tile_pool(name="psA", bufs=2, space="PSUM") as psA, \
             tc.tile_pool(name="psB", bufs=1, space="PSUM") as psB, \
             tc.tile_pool(name="psC", bufs=2, space="PSUM") as psC, \
             tc.tile_pool(name="psTR", bufs=1, space="PSUM") as psTR:

            # ---------- constants ----------
            idf = pe.tile([128, 128], F32, tag="idf")
            make_identity(nc, idf[:])
            id32 = idf[0:32, 0:32]
            ones_f = pe.tile([1, 128], F32, tag="ones_f")
            nc.gpsimd.memset(ones_f[:], 1.0)
            zero_f = pe.tile([128, 128], F32, tag="zero_f")
            nc.gpsimd.memset(zero_f[:], 0.0)
            ones_b = pe.tile([1, 128], BF16, tag="ones_b")
            nc.gpsimd.tensor_copy(ones_b[:], ones_f[:])
            idb = pe.tile([128, 128], BF16, tag="idb")
            nc.gpsimd.tensor_copy(idb[:], idf[:])
            idb32 = idb[0:32, 0:32]

            def load_b(pool, dram_ap, rows, cols, tag):
                """Direct DMA of a host-pre-cast bf16 weight into SBUF."""
                r = pool.tile([rows, cols], BF16, tag=tag)
                nc.sync.dma_start(r[:], dram_ap)
                return r

            # ---------- encoder weights (bf16, direct DMA) ----------
            wih_r = load_b(wenc, d_wih[:], 66, G, "wih")
            whh_r = [load_b(wenc, d_whh[128 * c:128 * (c + 1), :], 128, G,
                            f"whh{c}") for c in range(4)]

            # ---------- encoder state ----------
            hT = pe.tile([128, H], BF16, tag="hT0")
            for c in range(4):
                nc.vector.tensor_copy(hT[:, 128 * c:128 * (c + 1)], zero_f[:])
            h_bh = pe.tile([BE, H], F32, tag="h0")
            nc.gpsimd.memset(h_bh[:], 0.0)

            # ---------- middle/decoder weights (bf16, direct DMA) ----------
            bias_b = load_b(pe, d_biasb[:], 1, 3584, "bias_b")
            bias_r = load_b(pe, d_biasr[:], 1, 2624, "bias_r")

            dwyc = wdec.tile([96, G], BF16, tag="dwyc")
            nc.sync.dma_start(dwyc[0:NY, :], d_dwy[:])
            # identity block staged at partitions 64:96 (for ypT_aug rows)
            id_hi_f = pe.tile([96, 32], F32, tag="id_hi_f")
            nc.sync.dma_start(id_hi_f[64:96, :], idf[0:32, 0:32])
            dwhh_r = [load_b(wdec, d_dwhh[128 * c:128 * (c + 1), :], 128, G,
                             f"dwhh{c}") for c in range(4)]
            dm1_r = [load_b(wdec, d_dm1[128 * c:128 * (c + 1), :], 128, M1,
                            f"dm1_{c}") for c in range(4)]
            dm2_r = [load_b(wdec, d_dm2[128 * c:128 * (c + 1), :], 128, M2,
                            f"dm2_{c}") for c in range(8)]
            dow_r = [load_b(wdec, d_dow[128 * c:128 * (c + 1), :], 128, NY,
                            f"dow_{c}") for c in range(4)]
            em1_r = [load_b(wdec, d_em1[128 * c:128 * (c + 1), :], 128, M1,
                            f"em1_{c}") for c in range(8)]
            em2_r = [load_b(wdec, d_em2[128 * c:128 * (c + 1), :], 128, M2,
                            f"em2_{c}") for c in range(8)]
            eow_r = [load_b(wdec, d_eow[128 * c:128 * (c + 1), :], 128, H,
                            f"eow_{c}") for c in range(4)]
            dcw_r = [load_b(wdec, d_dcw[128 * c:128 * (c + 1), :], 128, G,
                            f"dcw_{c}") for c in range(8)]
            xlast_r = pe.tile([NX, BD], BF16, tag="xlastr")
            nc.sync.dma_start(xlast_r[:], d_xlast[:])

            # ---------- encode loop ----------
            for t in range(et):
                xs = xsp.tile([66, 128], BF16, tag="xs_r")
                nc.sync.dma_start(xs[:], d_xin[:, t * BE:(t + 1) * BE])

                g1a = psA.tile([BE, 512], F32, tag="gA1")
                g1b = psA.tile([BE, 512], F32, tag="gA2")
                g2 = psB.tile([BE, 512], F32, tag="gB")
                g3 = psC.tile([BE, 512], F32, tag="gC")
                # bank-grouped: finish each PSUM bank before switching
                nc.tensor.matmul(g1a[:], xs[:], wih_r[:, 0:512],
                                 start=True, stop=False)
                for c in range(4):
                    nc.tensor.matmul(g1a[:], hT[:, 128 * c:128 * (c + 1)],
                                     whh_r[c][:, 0:512],
                                     start=False, stop=(c == 3))
                nc.tensor.matmul(g1b[:], xs[:], wih_r[:, 512:1024],
                                 start=True, stop=False)
                for c in range(4):
                    nc.tensor.matmul(g1b[:], hT[:, 128 * c:128 * (c + 1)],
                                     whh_r[c][:, 512:1024],
                                     start=False, stop=(c == 3))
                for c in range(4):
                    nc.tensor.matmul(g2[:], hT[:, 128 * c:128 * (c + 1)],
                                     whh_r[c][:, 1024:1536],
                                     start=(c == 0), stop=False)
                nc.tensor.matmul(g2[:], ones_b[0:1, 0:BE],
                                 bias_r[0:1, 0:512], start=False, stop=True)
                nc.tensor.matmul(g3[:], xs[:], wih_r[:, 1024:1536],
                                 start=True, stop=True)

                r_t = tp.tile([BE, 512], F32, tag="r")
                z_t = tp.tile([BE, 512], F32, tag="z")
                omz = tp.tile([BE, 512], F32, tag="omz")
                nc.scalar.activation(r_t[:], g1a[:], AF.Sigmoid)
                nc.scalar.activation(z_t[:], g1b[:], AF.Sigmoid)
                nc.scalar.activation(omz[:], g1b[:], AF.Sigmoid,
                                     scale=-1.0)
                rhn = tp.tile([BE, 512], F32, tag="rhn")
                nc.vector.tensor_mul(rhn[:], r_t[:], g2[:])
                npre = tp.tile([BE, 512], F32, tag="npre")
                nc.vector.tensor_add(npre[:], rhn[:], g3[:])
                n_t = tp.tile([BE, 512], F32, tag="n")
                nc.scalar.activation(n_t[:], npre[:], AF.Tanh)
                a_t = tp.tile([BE, 512], F32, tag="a")
                nc.vector.tensor_mul(a_t[:], omz[:], n_t[:])
                b_t = tp.tile([BE, 512], F32, tag="b")
                nc.gpsimd.tensor_mul(b_t[:], z_t[:], h_bh[:])
                h_new = st.tile([BE, H], F32, tag="h")
                nc.vector.tensor_add(h_new[:], a_t[:], b_t[:])
                h_nb = st.tile([BE, H], BF16, tag="hb")
                nc.gpsimd.tensor_copy(h_nb[:], h_new[:])

                ptr = psTR.tile([128, 512], BF16, tag="tr")
                for c in range(4):
                    nc.tensor.transpose(ptr[:, 128 * c:128 * (c + 1)],
                                        h_nb[:, 128 * c:128 * (c + 1)],
                                        idb[:])
                hT_new = st.tile([128, H], BF16, tag="hT")
                nc.vector.tensor_copy(hT_new[:], ptr[:])
                hT, h_bh = hT_new, h_new

            # ---------- reshard: AllToAll of 16-row slices ----------
            nc.sync.dma_start(cc_in[:], h_bh[:])
            nc.gpsimd.collective_compute(
                "AllToAll", OP.bypass,
                replica_groups=[list(range(NCORE))],
                ins=[cc_in[:]], outs=[cc_out[:]])

            pxa = md.tile([BD, H], F32, tag="pA")
            pxb = md.tile([BD, H], F32, tag="pB")
            pya = md.tile([BD, H], F32, tag="pA")
            pyb = md.tile([BD, H], F32, tag="pB")
            nc.sync.dma_start(pxa[0:16, :], cc_out[0][:])
            nc.sync.dma_start(pxa[16:32, :], cc_out[1][:])
            nc.sync.dma_start(pxb[0:16, :], cc_out[2][:])
            nc.sync.dma_start(pxb[16:32, :], cc_out[3][:])
            nc.sync.dma_start(pya[0:16, :], cc_out[4][:])
            nc.sync.dma_start(pya[16:32, :], cc_out[5][:])
            nc.sync.dma_start(pyb[0:16, :], cc_out[6][:])
            nc.sync.dma_start(pyb[16:32, :], cc_out[7][:])
            hx = md.tile([BD, H], BF16, tag="hx")
            hy = md.tile([BD, H], BF16, tag="hy")
            nc.vector.tensor_add(hx[:], pxa[:], pxb[:])
            nc.vector.tensor_add(hy[:], pya[:], pyb[:])

            def trsp_b(srcb, cols, tag):
                """srcb [BD, cols] bf16 -> bf16 [128, (cols//128)*BD]."""
                nch = cols // 128
                p = psTR.tile([128, nch * BD], BF16, tag="tr")
                for c in range(nch):
                    nc.tensor.transpose(p[:, BD * c:BD * (c + 1)],
                                        srcb[:, 128 * c:128 * (c + 1)], idb32)
                o = pe.tile([128, nch * BD], BF16, tag=tag)
                nc.scalar.copy(o[:], p[:])
                return o

            hxT = trsp_b(hx, H, "hxT")
            hyT = trsp_b(hy, H, "hyT")

            m1a = psA.tile([BD, 512], F32, tag="gA1")
            m1b = psA.tile([BD, 512], F32, tag="gA2")
            for c in range(8):
                wt = em1_r[c]
                s = (hxT if c < 4 else hyT)[:, BD * (c % 4):BD * (c % 4 + 1)]
                nc.tensor.matmul(m1a[:], s, wt[:, 0:512],
                                 start=(c == 0), stop=False)
                nc.tensor.matmul(m1b[:], s, wt[:, 512:1024],
                                 start=(c == 0), stop=False)
            nc.tensor.matmul(m1a[:], ones_b[0:1, 0:BD],
                             bias_b[0:1, 1536:2048], start=False, stop=True)
            nc.tensor.matmul(m1b[:], ones_b[0:1, 0:BD],
                             bias_b[0:1, 2048:2560], start=False, stop=True)
            hm1 = tq.tile([BD, M1], BF16, tag="hm1m")
            nc.scalar.activation(hm1[:, 0:512], m1a[:], AF.Relu)
            nc.scalar.activation(hm1[:, 512:1024], m1b[:], AF.Relu)
            hm1T = trsp_b(hm1, M1, "hm1T_m")

            m2 = psB.tile([BD, M2], F32, tag="gB")
            for c in range(8):
                nc.tensor.matmul(m2[:], hm1T[:, BD * c:BD * (c + 1)],
                                 em2_r[c][:], start=(c == 0), stop=False)
            nc.tensor.matmul(m2[:], ones_b[0:1, 0:BD], bias_b[0:1, 2560:3072],
                             start=False, stop=True)
            hm2 = tq.tile([BD, M2], BF16, tag="hm2m")
            nc.scalar.activation(hm2[:], m2[:], AF.Relu)
            hm2T = trsp_b(hm2, M2, "hm2T_m")

            zp = psC.tile([BD, H], F32, tag="gC")
            for c in range(4):
                nc.tensor.matmul(zp[:], hm2T[:, BD * c:BD * (c + 1)],
                                 eow_r[c][:], start=(c == 0), stop=False)
            nc.tensor.matmul(zp[:], ones_b[0:1, 0:BD], bias_b[0:1, 3072:3584],
                             start=False, stop=True)
            z_sb = md.tile([BD, H], BF16, tag="z_sb")
            nc.scalar.copy(z_sb[:], zp[:])
            zT = trsp_b(z_sb, H, "zT")

            # const = cat(h_x, z) @ d_Wih[:, :2H].T + d_bih + d_bhh(r,z)
            cpa = psA.tile([96, 512], F32, tag="gA1")
            cpa = cpa[64:96, :]
            cpb = psA.tile([96, 512], F32, tag="gA2")
            cpb = cpb[64:96, :]
            cpn = psB.tile([96, 512], F32, tag="gB")
            cpn = cpn[64:96, :]
            for c in range(8):
                wt = dcw_r[c]
                s = (hxT if c < 4 else zT)[:, BD * (c % 4):BD * (c % 4 + 1)]
                nc.tensor.matmul(cpa[:], s, wt[:, 0:512],
                                 start=(c == 0), stop=False)
                nc.tensor.matmul(cpb[:], s, wt[:, 512:1024],
                                 start=(c == 0), stop=False)
                nc.tensor.matmul(cpn[:], s, wt[:, 1024:1536],
                                 start=(c == 0), stop=False)
            nc.tensor.matmul(cpa[:], ones_b[0:1, 0:BD],
                             bias_b[0:1, 0:512], start=False, stop=True)
            nc.tensor.matmul(cpb[:], ones_b[0:1, 0:BD],
                             bias_b[0:1, 512:1024], start=False, stop=True)
            nc.tensor.matmul(cpn[:], ones_b[0:1, 0:BD],
                             bias_b[0:1, 1024:1536], start=False, stop=True)
            nc.vector.tensor_copy(dwyc[64:96, 0:512], cpa[:])
            nc.vector.tensor_copy(dwyc[64:96, 512:1024], cpb[:])
            nc.vector.tensor_copy(dwyc[64:96, 1024:1536], cpn[:])

            # decoder init
            hdT = st.tile([128, 4 * BD], BF16, tag="hdT")
            nc.vector.tensor_copy(hdT[:], zero_f[:])
            hd = st.tile([BD, H], F32, tag="hd")
            nc.gpsimd.memset(hd[:], 0.0)
            ypT = st.tile([96, BD], BF16, tag="ypT")
            nc.vector.tensor_copy(ypT[0:NX, :], xlast_r[:])
            nc.vector.tensor_copy(ypT[64:96, :], id_hi_f[64:96, :])

            # ---------- decode loop ----------
            for t in range(hor):
                g1a = psA.tile([BD, 512], F32, tag="gA1")
                g1b = psA.tile([BD, 512], F32, tag="gA2")
                g2 = psB.tile([BD, 512], F32, tag="gB")
                g3 = psC.tile([BD, 512], F32, tag="gC")
                # h-side first: depends only on hdT (ready since last GRU
                # phase), so these stream during the previous step's MLP.
                # The yp/const matmuls close each group once ypT lands.
                for c in range(4):
                    nc.tensor.matmul(g1a[:], hdT[:, BD * c:BD * (c + 1)],
                                     dwhh_r[c][:, 0:512],
                                     start=(c == 0), stop=False)
                for c in range(4):
                    nc.tensor.matmul(g2[:], hdT[:, BD * c:BD * (c + 1)],
                                     dwhh_r[c][:, 1024:1536],
                                     start=(c == 0), stop=False)
                nc.tensor.matmul(g2[:], ones_b[0:1, 0:BD],
                                 bias_r[0:1, 512:1024], start=False, stop=True)
                for c in range(4):
                    nc.tensor.matmul(g1b[:], hdT[:, BD * c:BD * (c + 1)],
                                     dwhh_r[c][:, 512:1024],
                                     start=(c == 0), stop=False)
                nc.tensor.matmul(g1a[:], ypT[:], dwyc[:, 0:512],
                                 start=False, stop=True)
                nc.tensor.matmul(g3[:], ypT[:], dwyc[:, 1024:1536],
                                 start=True, stop=True)
                nc.tensor.matmul(g1b[:], ypT[:], dwyc[:, 512:1024],
                                 start=False, stop=True)

                r_t = tp.tile([BD, 512], F32, tag="r")
                z_t = tp.tile([BD, 512], F32, tag="z")
                omz = tp.tile([BD, 512], F32, tag="omz")
                nc.scalar.activation(r_t[:], g1a[:], AF.Sigmoid)
                nc.scalar.activation(z_t[:], g1b[:], AF.Sigmoid)
                nc.scalar.activation(omz[:], g1b[:], AF.Sigmoid,
                                     scale=-1.0)
                rhn = tp.tile([BD, 512], F32, tag="rhn")
                nc.vector.tensor_mul(rhn[:], r_t[:], g2[:])
                npre = tp.tile([BD, 512], F32, tag="npre")
                nc.vector.tensor_add(npre[:], rhn[:], g3[:])
                n_t = tp.tile([BD, 512], F32, tag="n")
                nc.scalar.activation(n_t[:], npre[:], AF.Tanh)
                a_t = tp.tile([BD, 512], F32, tag="a")
                nc.vector.tensor_mul(a_t[:], omz[:], n_t[:])
                b_t = tp.tile([BD, 512], F32, tag="b")
                nc.gpsimd.tensor_mul(b_t[:], z_t[:], hd[:])
                hd_new = st.tile([BD, H], F32, tag="hd")
                nc.vector.tensor_add(hd_new[:], a_t[:], b_t[:])
                hd_nb = st.tile([BD, H], BF16, tag="hdb")
                nc.gpsimd.tensor_copy(hd_nb[:], hd_new[:])

                ptr = psTR.tile([128, 4 * BD], BF16, tag="tr")
                for c in range(4):
                    nc.tensor.transpose(ptr[:, BD * c:BD * (c + 1)],
                                        hd_nb[:, 128 * c:128 * (c + 1)],
                                        idb32)
                hdT_new = st.tile([128, 4 * BD], BF16, tag="hdT")
                nc.vector.tensor_copy(hdT_new[:], ptr[:])
                hdT, hd = hdT_new, hd_new

                m1a = psA.tile([BD, 512], F32, tag="gA1")
                m1b = psA.tile([BD, 512], F32, tag="gA2")
                for c in range(4):
                    nc.tensor.matmul(m1a[:], hdT[:, BD * c:BD * (c + 1)],
                                     dm1_r[c][:, 0:512],
                                     start=(c == 0), stop=False)
                nc.tensor.matmul(m1a[:], ones_b[0:1, 0:BD],
                                 bias_r[0:1, 1024:1536], start=False, stop=True)
                hm1 = tq.tile([BD, M1], BF16, tag="hm1")
                nc.scalar.activation(hm1[:, 0:512], m1a[:], AF.Relu)
                for c in range(4):
                    nc.tensor.matmul(m1b[:],
                                     hdT[:, BD * c:BD * (c + 1)],
                                     dm1_r[c][:, 512:1024],
                                     start=(c == 0), stop=False)
                nc.tensor.matmul(m1b[:], ones_b[0:1, 0:BD],
                                 bias_r[0:1, 1536:2048], start=False, stop=True)
                nc.scalar.activation(hm1[:, 512:1024], m1b[:], AF.Relu)
                hm1Ta = tq.tile([128, 4 * BD], BF16, tag="hm1Ta")
                hm1Tb = tq.tile([128, 4 * BD], BF16, tag="hm1Tb")
                p1 = psTR.tile([128, 4 * BD], BF16, tag="tr")
                for c in range(4):
                    nc.tensor.transpose(p1[:, BD * c:BD * (c + 1)],
                                        hm1[:, 128 * c:128 * (c + 1)], idb32)
                nc.vector.tensor_copy(hm1Ta[:], p1[:])
                p1b = psTR.tile([128, 4 * BD], BF16, tag="tr")
                for c in range(4):
                    nc.tensor.transpose(p1b[:, BD * c:BD * (c + 1)],
                                        hm1[:, 512 + 128 * c:640 + 128 * c],
                                        idb32)
                nc.vector.tensor_copy(hm1Tb[:], p1b[:])

                m2 = psB.tile([BD, M2], F32, tag="gB")
                for c in range(8):
                    s = (hm1Ta if c < 4 else hm1Tb)[:, BD * (c % 4):
                                                    BD * (c % 4 + 1)]
                    nc.tensor.matmul(m2[:], s, dm2_r[c][:],
                                     start=(c == 0), stop=False)
                nc.tensor.matmul(m2[:], ones_b[0:1, 0:BD],
                                 bias_r[0:1, 2048:2560], start=False, stop=True)
                hm2 = tq.tile([BD, M2], BF16, tag="hm2")
                nc.scalar.activation(hm2[:], m2[:], AF.Relu)
                p2 = psTR.tile([128, 4 * BD], BF16, tag="tr")
                for c in range(4):
                    nc.tensor.transpose(p2[:, BD * c:BD * (c + 1)],
                                        hm2[:, 128 * c:128 * (c + 1)], idb32)
                hm2T = tq.tile([128, 4 * BD], BF16, tag="hm2T")
                nc.vector.tensor_copy(hm2T[:], p2[:])

                yp_ps = psC.tile([BD, NY], F32, tag="gC")
                for c in range(4):
                    nc.tensor.matmul(yp_ps[:], hm2T[:, BD * c:BD * (c + 1)],
                                     dow_r[c][:], start=(c == 0), stop=False)
                nc.tensor.matmul(yp_ps[:], ones_b[0:1, 0:BD],
                                 bias_r[0:1, 2560:2624],
                                 start=False, stop=True)
                y_sb = tp.tile([BD, NY], F32, tag="y_sb")
                nc.scalar.copy(y_sb[:], yp_ps[:])
                nc.sync.dma_start(d_out[:, NY * t:NY * (t + 1)], y_sb[:])
                if t + 1 < hor:
                    y_b = tp.tile([BD, NY], BF16, tag="y_b")
                    nc.vector.tensor_copy(y_b[:], yp_ps[:])
                    p3 = psTR.tile([NX, BD], BF16, tag="tr")
                    nc.tensor.transpose(p3[:], y_b[:], idb32)
                    ypT_new = st.tile([96, BD], BF16, tag="ypT")
                    nc.vector.tensor_copy(ypT_new[0:NX, :], p3[:])
                    nc.vector.tensor_copy(ypT_new[64:96, :],
                                          id_hi_f[64:96, :])
                    ypT = ypT_new

    nc.compile()
    return nc


# ---------------------------------------------------------------------------
# Host-side sharding
# ---------------------------------------------------------------------------

def shard_inputs(inp, et=100, x_real=50, hor=60):
    f32 = np.float32
    x, y = np.asarray(inp["x"], f32), np.asarray(inp["y"], f32)
    tx = x.shape[1]
    chains = [("xf", False, x), ("xb", True, x),
              ("ef", False, y), ("eb", True, y)]
    in_maps = []
    shared = {}

    def bcast(a):
        return np.ascontiguousarray(a).astype(BF16NP)

    def wih_aug(pre):
        wih = np.asarray(inp[pre + "_Wih"], f32)
        bih = np.asarray(inp[pre + "_bih"], f32)
        bhh = np.asarray(inp[pre + "_bhh"], f32)
        aug = np.zeros((66, G), f32)
        aug[0:64, :] = wih.T
        bias = bih.copy()
        bias[0:2 * H] += bhh[0:2 * H]
        aug[64, :] = bias
        aug[65, H:2 * H] = BIG
        return bcast(aug)

    d_Wih = np.asarray(inp["d_Wih"], f32)
    d_bih = np.asarray(inp["d_bih"], f32)
    d_bhh = np.asarray(inp["d_bhh"], f32)
    dc_b = d_bih.copy()
    dc_b[0:2 * H] += d_bhh[0:2 * H]

    shared["em_w1t"] = bcast(np.asarray(inp["em_W1"], f32).T)
    shared["em_w2t"] = bcast(np.asarray(inp["em_W2"], f32).T)
    shared["eo_wt"] = bcast(np.asarray(inp["eo_W"], f32).T)
    shared["dc_wt"] = bcast(d_Wih[:, 0:2 * H].T)
    bias_b = np.concatenate([
        dc_b, np.asarray(inp["em_b1"], f32), np.asarray(inp["em_b2"], f32),
        np.asarray(inp["eo_b"], f32)])
    shared["bias_b"] = bcast(bias_b[None, :])
    shared["dwy_t"] = bcast(d_Wih[:, 2 * H:].T)
    shared["dwhh_t"] = bcast(np.asarray(inp["d_Whh"], f32).T)
    shared["dm_w1t"] = bcast(np.asarray(inp["dm_W1"], f32).T)
    shared["dm_w2t"] = bcast(np.asarray(inp["dm_W2"], f32).T)
    shared["do_wt"] = bcast(np.asarray(inp["do_W"], f32).T)
    bias_r_base = np.concatenate([
        np.zeros(H, f32), d_bhh[2 * H:], np.asarray(inp["dm_b1"], f32),
        np.asarray(inp["dm_b2"], f32), np.asarray(inp["do_b"], f32)])

    for j in range(NCORE):
        chain, half = j // 2, j % 2
        pre, rev, seq = chains[chain]
        T = seq.shape[1]
        s = seq[128 * half:128 * (half + 1)]          # [128, T, 64]
        xin = np.zeros((66, et, BE), f32)
        xin[64, :, :] = 1.0
        pad = et - T
        if pad:
            xin[65, 0:pad, :] = 1.0
        order = np.arange(T)[::-1] if rev else np.arange(T)
        xin[0:64, pad:, :] = s[:, order, :].transpose(2, 1, 0)
        m = dict(shared)
        m["xin"] = bcast(xin.reshape(66, et * BE))
        m["wih_aug"] = wih_aug(pre)
        m["whh_t"] = bcast(np.asarray(inp[pre + "_Whh"], f32).T)
        bias_r = bias_r_base.copy()
        bias_r[0:H] = np.asarray(inp[pre + "_bhh"], f32)[2 * H:]
        m["bias_r"] = bcast(bias_r[None, :])
        xl = np.concatenate([x[16 * j:16 * j + 16, -1, :],
                             x[128 + 16 * j:128 + 16 * j + 16, -1, :]])
        m["xlast_t"] = bcast(xl.T)
        in_maps.append(m)
    return in_maps


def unshard(results, hor=60):
    out = np.zeros((B, hor, NY), np.float32)
    for j in range(NCORE):
        o = results[j]["out"].reshape(BD, hor, NY)
        out[16 * j:16 * j + 16] = o[0:16]
        out[128 + 16 * j:128 + 16 * j + 16] = o[16:32]
    return out


_NC = None


def kernel(**inputs):
    global _NC
    from concourse.bass_utils import run_bass_kernel_spmd
    if _NC is None:
        _NC = build_nc()
    in_maps = shard_inputs(inputs)
    res = run_bass_kernel_spmd(_NC, in_maps, core_ids=list(range(NCORE)))
    return unshard(res.results)


# revision 42
# speedup vs baseline: 1.1632x; 1.1632x over previous
"""Trainium2 Bass kernel for the GRU autoencoder.

Distribution strategy (8 NeuronCores):
  Encode : chain-parallel x batch-parallel. Core j handles GRU chain j//2
           (xf, xb, ef, eb) on batch half j%2 (128 rows), running a uniform
           100-step loop. The 50-step x-chains get 50 leading exact identity
           steps (update-gate pre-activation forced to +BIG => z=1 => h'=h).
           Input-side gates, biases and the pad flag ride the same matmul via
           augmented stationary rows (features + ones-row + flag-row).
  Reshard: AllToAll moves 16-row slices so every core assembles the hidden
           states for its own 32-row decode shard at *fixed* (SPMD-uniform)
           indices: core j decodes global rows [16j:16j+16] u [128+16j:+16].
  Middle : per-shard MLP (em1/em2/eo) + decoder const precompute (bf16).
  Decode : 60 autoregressive steps per core on its 32-row shard.

All weights (the matmul MOVING operands) are bf16: the PE streams 16-bit
moving data at 1 col/cycle vs 2 for fp32r, so this halves GEMM time.
Stationary operands (activations) stay fp32r for precision. Weights are
cast to bf16 on the host, halving DMA and removing on-device cast work.

PSUM budget (8 banks): gA 2 | gB 2x1 | gC 2x1 | tr 2x1.
"""

import sys

sys.path.insert(0, "/opt/trn_rl_repo")

import numpy as np
import ml_dtypes

import concourse.bass as bass
import concourse.mybir as mybir
import concourse.tile as tile
from concourse import bacc
from concourse.masks import make_identity

dt = mybir.dt
AF = mybir.ActivationFunctionType
OP = mybir.AluOpType

B, TX, TY, NX, NY, H, HOR = 256, 50, 100, 64, 64, 512, 60
M1, M2 = 1024, 512
G = 3 * H
NCORE = 8
BE = 128   # encoder batch rows per core
BD = 32    # decoder batch rows per core
BIG = 30000.0

F32, F32R, BF16 = dt.float32, dt.float32r, dt.bfloat16
BF16NP = ml_dtypes.bfloat16


def build_nc(et=100, x_real=50, hor=60):
    nc = bacc.Bacc("TRN2", target_bir_lowering=False, debug=False,
                   num_devices=NCORE)

    # ---- DRAM parameters (identical names on every core; content differs) --
    d_xin = nc.dram_tensor("xin", [66, et * BE], BF16, kind="ExternalInput")
    d_wih = nc.dram_tensor("wih_aug", [66, G], BF16, kind="ExternalInput")
    d_whh = nc.dram_tensor("whh_t", [H, G], BF16, kind="ExternalInput")

    d_em1 = nc.dram_tensor("em_w1t", [2 * H, M1], BF16, kind="ExternalInput")
    d_em2 = nc.dram_tensor("em_w2t", [M1, M2], BF16, kind="ExternalInput")
    d_eow = nc.dram_tensor("eo_wt", [M2, H], BF16, kind="ExternalInput")
    d_dcw = nc.dram_tensor("dc_wt", [2 * H, G], BF16, kind="ExternalInput")
    # column-packed bf16 bias rows: dcb|em1b|em2b|eob (3584)
    d_biasb = nc.dram_tensor("bias_b", [1, 3584], BF16, kind="ExternalInput")

    d_dwy = nc.dram_tensor("dwy_t", [NY, G], BF16, kind="ExternalInput")
    d_dwhh = nc.dram_tensor("dwhh_t", [H, G], BF16, kind="ExternalInput")
    d_dm1 = nc.dram_tensor("dm_w1t", [H, M1], BF16, kind="ExternalInput")
    d_dm2 = nc.dram_tensor("dm_w2t", [M1, M2], BF16, kind="ExternalInput")
    d_dow = nc.dram_tensor("do_wt", [M2, NY], BF16, kind="ExternalInput")
    # column-packed bf16 bias rows: bhhn|dbhhn|dm1b|dm2b|dob (2624)
    d_biasr = nc.dram_tensor("bias_r", [1, 2624], BF16, kind="ExternalInput")
    d_xlast = nc.dram_tensor("xlast_t", [NX, BD], BF16, kind="ExternalInput")

    d_out = nc.dram_tensor("out", [BD, hor * NY], F32, kind="ExternalOutput")

    cc_in = nc.dram_tensor("cc_in", [BE, H], F32)
    cc_out = nc.dram_tensor("cc_out", [NCORE, 16, H], F32)

    with tile.TileContext(nc) as tc:
        with tc.tile_pool(name="wenc", bufs=1) as wenc, \
             tc.tile_pool(name="wdec", bufs=1) as wdec, \
             tc.tile_pool(name="state", bufs=2) as st, \
             tc.tile_pool(name="xsp", bufs=4) as xsp, \
             tc.tile_pool(name="tmp", bufs=2) as tp, \
             tc.tile_pool(name="tmp1", bufs=1) as tq, \
             tc.tile_pool(name="mid", bufs=1) as md, \
             tc.tile_pool(name="persist", bufs=1) as pe, \
             tc.tile_pool(name="psA", bufs=2, space="PSUM") as psA, \
             tc.tile_pool(name="psB", bufs=1, space="PSUM") as psB, \
             tc.tile_pool(name="psC", bufs=2, space="PSUM") as psC, \
             tc.tile_pool(name="psTR", bufs=1, space="PSUM") as psTR:

            # ---------- constants ----------
            idf = pe.tile([128, 128], F32, tag="idf")
            make_identity(nc, idf[:])
            id32 = idf[0:32, 0:32]
            ones_f = pe.tile([1, 128], F32, tag="ones_f")
            nc.gpsimd.memset(ones_f[:], 1.0)
            zero_f = pe.tile([128, 128], F32, tag="zero_f")
            nc.gpsimd.memset(zero_f[:], 0.0)
            ones_b = pe.tile([1, 128], BF16, tag="ones_b")
            nc.gpsimd.tensor_copy(ones_b[:], ones_f[:])
            idb = pe.tile([128, 128], BF16, tag="idb")
            nc.gpsimd.tensor_copy(idb[:], idf[:])
            idb32 = idb[0:32, 0:32]

            def load_b(pool, dram_ap, rows, cols, tag):
                """Direct DMA of a host-pre-cast bf16 weight into SBUF."""
                r = pool.tile([rows, cols], BF16, tag=tag)
                nc.sync.dma_start(r[:], dram_ap)
                return r

            # ---------- encoder weights (bf16, direct DMA) ----------
            wih_r = load_b(wenc, d_wih[:], 66, G, "wih")
            whh_r = [load_b(wenc, d_whh[128 * c:128 * (c + 1), :], 128, G,
                            f"whh{c}") for c in range(4)]

            # ---------- encoder state ----------
            hT = pe.tile([128, H], BF16, tag="hT0")
            for c in range(4):
                nc.vector.tensor_copy(hT[:, 128 * c:128 * (c + 1)], zero_f[:])
            h_bh = pe.tile([BE, H], F32, tag="h0")
            nc.gpsimd.memset(h_bh[:], 0.0)

            # ---------- middle/decoder weights (bf16, direct DMA) ----------
            bias_b = load_b(pe, d_biasb[:], 1, 3584, "bias_b")
            bias_r = load_b(pe, d_biasr[:], 1, 2624, "bias_r")

            dwyc = wdec.tile([96, G], BF16, tag="dwyc")
            nc.sync.dma_start(dwyc[0:NY, :], d_dwy[:])
            # identity block staged at partitions 64:96 (for ypT_aug rows)
            id_hi_f = pe.tile([96, 32], F32, tag="id_hi_f")
            nc.sync.dma_start(id_hi_f[64:96, :], idf[0:32, 0:32])
            dwhh_r = [load_b(wdec, d_dwhh[128 * c:128 * (c + 1), :], 128, G,
                             f"dwhh{c}") for c in range(4)]
            dm1_r = [load_b(wdec, d_dm1[128 * c:128 * (c + 1), :], 128, M1,
                            f"dm1_{c}") for c in range(4)]
            dm2_r = [load_b(wdec, d_dm2[128 * c:128 * (c + 1), :], 128, M2,
                            f"dm2_{c}") for c in range(8)]
            dow_r = [load_b(wdec, d_dow[128 * c:128 * (c + 1), :], 128, NY,
                            f"dow_{c}") for c in range(4)]
            em1_r = [load_b(wdec, d_em1[128 * c:128 * (c + 1), :], 128, M1,
                            f"em1_{c}") for c in range(8)]
            em2_r = [load_b(wdec, d_em2[128 * c:128 * (c + 1), :], 128, M2,
                            f"em2_{c}") for c in range(8)]
            eow_r = [load_b(wdec, d_eow[128 * c:128 * (c + 1), :], 128, H,
                            f"eow_{c}") for c in range(4)]
            dcw_r = [load_b(wdec, d_dcw[128 * c:128 * (c + 1), :], 128, G,
                            f"dcw_{c}") for c in range(8)]
            xlast_r = pe.tile([NX, BD], BF16, tag="xlastr")
            nc.sync.dma_start(xlast_r[:], d_xlast[:])

            # ---------- encode loop ----------
            for t in range(et):
                xs = xsp.tile([66, 128], BF16, tag="xs_r")
                nc.sync.dma_start(xs[:], d_xin[:, t * BE:(t + 1) * BE])

                g1a = psA.tile([BE, 512], F32, tag="gA1")
                g1b = psA.tile([BE, 512], F32, tag="gA2")
                g2 = psB.tile([BE, 512], F32, tag="gB")
                g3 = psC.tile([BE, 512], F32, tag="gC")
                # bank-grouped: finish each PSUM bank before switching
                nc.tensor.matmul(g1a[:], xs[:], wih_r[:, 0:512],
                                 start=True, stop=False)
                for c in range(4):
                    nc.tensor.matmul(g1a[:], hT[:, 128 * c:128 * (c + 1)],
                                     whh_r[c][:, 0:512],
                                     start=False, stop=(c == 3))
                nc.tensor.matmul(g1b[:], xs[:], wih_r[:, 512:1024],
                                 start=True, stop=False)
                for c in range(4):
                    nc.tensor.matmul(g1b[:], hT[:, 128 * c:128 * (c + 1)],
                                     whh_r[c][:, 512:1024],
                                     start=False, stop=(c == 3))
                for c in range(4):
                    nc.tensor.matmul(g2[:], hT[:, 128 * c:128 * (c + 1)],
                                     whh_r[c][:, 1024:1536],
                                     start=(c == 0), stop=False)
                nc.tensor.matmul(g2[:], ones_b[0:1, 0:BE],
                                 bias_r[0:1, 0:512], start=False, stop=True)
                nc.tensor.matmul(g3[:], xs[:], wih_r[:, 1024:1536],
                                 start=True, stop=True)

                r_t = tp.tile([BE, 512], F32, tag="r")
                z_t = tp.tile([BE, 512], F32, tag="z")
                omz = tp.tile([BE, 512], F32, tag="omz")
                nc.scalar.activation(r_t[:], g1a[:], AF.Sigmoid)
                nc.scalar.activation(z_t[:], g1b[:], AF.Sigmoid)
                nc.scalar.activation(omz[:], g1b[:], AF.Sigmoid,
                                     scale=-1.0)
                rhn = tp.tile([BE, 512], F32, tag="rhn")
                nc.vector.tensor_mul(rhn[:], r_t[:], g2[:])
                npre = tp.tile([BE, 512], F32, tag="npre")
                nc.vector.tensor_add(npre[:], rhn[:], g3[:])
                n_t = tp.tile([BE, 512], F32, tag="n")
                nc.scalar.activation(n_t[:], npre[:], AF.Tanh)
                a_t = tp.tile([BE, 512], F32, tag="a")
                nc.vector.tensor_mul(a_t[:], omz[:], n_t[:])
                b_t = tp.tile([BE, 512], F32, tag="b")
                nc.gpsimd.tensor_mul(b_t[:], z_t[:], h_bh[:])
                # bf16 state for the PE path on DVE (critical); f32 state for
                # next step's z*h on gpsimd (off the critical path).
                h_nb = st.tile([BE, H], BF16, tag="hb")
                nc.vector.tensor_add(h_nb[:], a_t[:], b_t[:])
                h_new = st.tile([BE, H], F32, tag="h")
                nc.gpsimd.tensor_add(h_new[:], a_t[:], b_t[:])

                # transpose as a regular bf16 matmul (identity moving): no
                # PE transpose-mode switch, which stalls following matmuls.
                ptr = psTR.tile([128, 512], F32, tag="tr")
                for c in range(4):
                    nc.tensor.matmul(ptr[:, 128 * c:128 * (c + 1)],
                                     h_nb[:, 128 * c:128 * (c + 1)],
                                     idb[:], start=True, stop=True)
                hT_new = st.tile([128, H], BF16, tag="hT")
                nc.vector.tensor_copy(hT_new[:], ptr[:])
                hT, h_bh = hT_new, h_new

            # ---------- reshard: AllToAll of 16-row slices ----------
            nc.sync.dma_start(cc_in[:], h_bh[:])
            nc.gpsimd.collective_compute(
                "AllToAll", OP.bypass,
                replica_groups=[list(range(NCORE))],
                ins=[cc_in[:]], outs=[cc_out[:]])

            pxa = md.tile([BD, H], F32, tag="pA")
            pxb = md.tile([BD, H], F32, tag="pB")
            pya = md.tile([BD, H], F32, tag="pA")
            pyb = md.tile([BD, H], F32, tag="pB")
            nc.sync.dma_start(pxa[0:16, :], cc_out[0][:])
            nc.sync.dma_start(pxa[16:32, :], cc_out[1][:])
            nc.sync.dma_start(pxb[0:16, :], cc_out[2][:])
            nc.sync.dma_start(pxb[16:32, :], cc_out[3][:])
            nc.sync.dma_start(pya[0:16, :], cc_out[4][:])
            nc.sync.dma_start(pya[16:32, :], cc_out[5][:])
            nc.sync.dma_start(pyb[0:16, :], cc_out[6][:])
            nc.sync.dma_start(pyb[16:32, :], cc_out[7][:])
            hx = md.tile([BD, H], BF16, tag="hx")
            hy = md.tile([BD, H], BF16, tag="hy")
            nc.vector.tensor_add(hx[:], pxa[:], pxb[:])
            nc.vector.tensor_add(hy[:], pya[:], pyb[:])

            def trsp_b(srcb, cols, tag):
                """srcb [BD, cols] bf16 -> bf16 [128, (cols//128)*BD]."""
                nch = cols // 128
                p = psTR.tile([128, nch * BD], F32, tag="tr")
                for c in range(nch):
                    nc.tensor.matmul(p[:, BD * c:BD * (c + 1)],
                                     srcb[:, 128 * c:128 * (c + 1)], idb32,
                                     start=True, stop=True)
                o = pe.tile([128, nch * BD], BF16, tag=tag)
                nc.scalar.copy(o[:], p[:])
                return o

            hxT = trsp_b(hx, H, "hxT")
            hyT = trsp_b(hy, H, "hyT")

            m1a = psA.tile([BD, 512], F32, tag="gA1")
            m1b = psA.tile([BD, 512], F32, tag="gA2")
            for c in range(8):
                wt = em1_r[c]
                s = (hxT if c < 4 else hyT)[:, BD * (c % 4):BD * (c % 4 + 1)]
                nc.tensor.matmul(m1a[:], s, wt[:, 0:512],
                                 start=(c == 0), stop=False)
                nc.tensor.matmul(m1b[:], s, wt[:, 512:1024],
                                 start=(c == 0), stop=False)
            nc.tensor.matmul(m1a[:], ones_b[0:1, 0:BD],
                             bias_b[0:1, 1536:2048], start=False, stop=True)
            nc.tensor.matmul(m1b[:], ones_b[0:1, 0:BD],
                             bias_b[0:1, 2048:2560], start=False, stop=True)
            hm1 = tq.tile([BD, M1], BF16, tag="hm1m")
            nc.scalar.activation(hm1[:, 0:512], m1a[:], AF.Relu)
            nc.scalar.activation(hm1[:, 512:1024], m1b[:], AF.Relu)
            hm1T = trsp_b(hm1, M1, "hm1T_m")

            m2 = psB.tile([BD, M2], F32, tag="gB")
            for c in range(8):
                nc.tensor.matmul(m2[:], hm1T[:, BD * c:BD * (c + 1)],
                                 em2_r[c][:], start=(c == 0), stop=False)
            nc.tensor.matmul(m2[:], ones_b[0:1, 0:BD], bias_b[0:1, 2560:3072],
                             start=False, stop=True)
            hm2 = tq.tile([BD, M2], BF16, tag="hm2m")
            nc.scalar.activation(hm2[:], m2[:], AF.Relu)
            hm2T = trsp_b(hm2, M2, "hm2T_m")

            zp = psC.tile([BD, H], F32, tag="gC")
            for c in range(4):
                nc.tensor.matmul(zp[:], hm2T[:, BD * c:BD * (c + 1)],
                                 eow_r[c][:], start=(c == 0), stop=False)
            nc.tensor.matmul(zp[:], ones_b[0:1, 0:BD], bias_b[0:1, 3072:3584],
                             start=False, stop=True)
            z_sb = md.tile([BD, H], BF16, tag="z_sb")
            nc.scalar.copy(z_sb[:], zp[:])
            zT = trsp_b(z_sb, H, "zT")

            # const = cat(h_x, z) @ d_Wih[:, :2H].T + d_bih + d_bhh(r,z)
            cpa = psA.tile([96, 512], F32, tag="gA1")
            cpa = cpa[64:96, :]
            cpb = psA.tile([96, 512], F32, tag="gA2")
            cpb = cpb[64:96, :]
            cpn = psB.tile([96, 512], F32, tag="gB")
            cpn = cpn[64:96, :]
            for c in range(8):
                wt = dcw_r[c]
                s = (hxT if c < 4 else zT)[:, BD * (c % 4):BD * (c % 4 + 1)]
                nc.tensor.matmul(cpa[:], s, wt[:, 0:512],
                                 start=(c == 0), stop=False)
                nc.tensor.matmul(cpb[:], s, wt[:, 512:1024],
                                 start=(c == 0), stop=False)
                nc.tensor.matmul(cpn[:], s, wt[:, 1024:1536],
                                 start=(c == 0), stop=False)
            nc.tensor.matmul(cpa[:], ones_b[0:1, 0:BD],
                             bias_b[0:1, 0:512], start=False, stop=True)
            nc.tensor.matmul(cpb[:], ones_b[0:1, 0:BD],
                             bias_b[0:1, 512:1024], start=False, stop=True)
            nc.tensor.matmul(cpn[:], ones_b[0:1, 0:BD],
                             bias_b[0:1, 1024:1536], start=False, stop=True)
            nc.vector.tensor_copy(dwyc[64:96, 0:512], cpa[:])
            nc.vector.tensor_copy(dwyc[64:96, 512:1024], cpb[:])
            nc.vector.tensor_copy(dwyc[64:96, 1024:1536], cpn[:])

            # decoder init
            hdT = st.tile([128, 4 * BD], BF16, tag="hdT")
            nc.vector.tensor_copy(hdT[:], zero_f[:])
            hd = st.tile([BD, H], F32, tag="hd")
            nc.gpsimd.memset(hd[:], 0.0)
            ypT = st.tile([96, BD], BF16, tag="ypT")
            nc.vector.tensor_copy(ypT[0:NX, :], xlast_r[:])
            nc.vector.tensor_copy(ypT[64:96, :], id_hi_f[64:96, :])

            # ---------- decode loop ----------
            for t in range(hor):
                g1a = psA.tile([BD, 512], F32, tag="gA1")
                g1b = psA.tile([BD, 512], F32, tag="gA2")
                g2 = psB.tile([BD, 512], F32, tag="gB")
                g3 = psC.tile([BD, 512], F32, tag="gC")
                # h-side first: depends only on hdT (ready since last GRU
                # phase), so these stream during the previous step's MLP.
                # The yp/const matmuls close each group once ypT lands.
                for c in range(4):
                    nc.tensor.matmul(g1a[:], hdT[:, BD * c:BD * (c + 1)],
                                     dwhh_r[c][:, 0:512],
                                     start=(c == 0), stop=False)
                for c in range(4):
                    nc.tensor.matmul(g2[:], hdT[:, BD * c:BD * (c + 1)],
                                     dwhh_r[c][:, 1024:1536],
                                     start=(c == 0), stop=False)
                nc.tensor.matmul(g2[:], ones_b[0:1, 0:BD],
                                 bias_r[0:1, 512:1024], start=False, stop=True)
                for c in range(4):
                    nc.tensor.matmul(g1b[:], hdT[:, BD * c:BD * (c + 1)],
                                     dwhh_r[c][:, 512:1024],
                                     start=(c == 0), stop=False)
                nc.tensor.matmul(g1a[:], ypT[:], dwyc[:, 0:512],
                                 start=False, stop=True)
                nc.tensor.matmul(g3[:], ypT[:], dwyc[:, 1024:1536],
                                 start=True, stop=True)
                nc.tensor.matmul(g1b[:], ypT[:], dwyc[:, 512:1024],
                                 start=False, stop=True)

                r_t = tp.tile([BD, 512], F32, tag="r")
                z_t = tp.tile([BD, 512], F32, tag="z")
                omz = tp.tile([BD, 512], F32, tag="omz")
                nc.scalar.activation(r_t[:], g1a[:], AF.Sigmoid)
                nc.scalar.activation(z_t[:], g1b[:], AF.Sigmoid)
                nc.scalar.activation(omz[:], g1b[:], AF.Sigmoid,
                                     scale=-1.0)
                rhn = tp.tile([BD, 512], F32, tag="rhn")
                nc.vector.tensor_mul(rhn[:], r_t[:], g2[:])
                npre = tp.tile([BD, 512], F32, tag="npre")
                nc.vector.tensor_add(npre[:], rhn[:], g3[:])
                n_t = tp.tile([BD, 512], F32, tag="n")
                nc.scalar.activation(n_t[:], npre[:], AF.Tanh)
                a_t = tp.tile([BD, 512], F32, tag="a")
                nc.vector.tensor_mul(a_t[:], omz[:], n_t[:])
                b_t = tp.tile([BD, 512], F32, tag="b")
                nc.gpsimd.tensor_mul(b_t[:], z_t[:], hd[:])
                hd_nb = st.tile([BD, H], BF16, tag="hdb")
                nc.vector.tensor_add(hd_nb[:], a_t[:], b_t[:])
                hd_new = st.tile([BD, H], F32, tag="hd")
                nc.gpsimd.tensor_add(hd_new[:], a_t[:], b_t[:])

                ptr = psTR.tile([128, 4 * BD], F32, tag="tr")
                for c in range(4):
                    nc.tensor.matmul(ptr[:, BD * c:BD * (c + 1)],
                                     hd_nb[:, 128 * c:128 * (c + 1)],
                                     idb32, start=True, stop=True)
                hdT_new = st.tile([128, 4 * BD], BF16, tag="hdT")
                nc.vector.tensor_copy(hdT_new[:], ptr[:])
                hdT, hd = hdT_new, hd_new

                m1a = psA.tile([BD, 512], F32, tag="gA1")
                m1b = psA.tile([BD, 512], F32, tag="gA2")
                for c in range(4):
                    nc.tensor.matmul(m1a[:], hdT[:, BD * c:BD * (c + 1)],
                                     dm1_r[c][:, 0:512],
                                     start=(c == 0), stop=False)
                nc.tensor.matmul(m1a[:], ones_b[0:1, 0:BD],
                                 bias_r[0:1, 1024:1536], start=False, stop=True)
                hm1 = tq.tile([BD, M1], BF16, tag="hm1")
                nc.scalar.activation(hm1[:, 0:512], m1a[:], AF.Relu)
                for c in range(4):
                    nc.tensor.matmul(m1b[:],
                                     hdT[:, BD * c:BD * (c + 1)],
                                     dm1_r[c][:, 512:1024],
                                     start=(c == 0), stop=False)
                nc.tensor.matmul(m1b[:], ones_b[0:1, 0:BD],
                                 bias_r[0:1, 1536:2048], start=False, stop=True)
                nc.scalar.activation(hm1[:, 512:1024], m1b[:], AF.Relu)
                hm1Ta = tq.tile([128, 4 * BD], BF16, tag="hm1Ta")
                hm1Tb = tq.tile([128, 4 * BD], BF16, tag="hm1Tb")
                p1 = psTR.tile([128, 4 * BD], F32, tag="tr")
                for c in range(4):
                    nc.tensor.matmul(p1[:, BD * c:BD * (c + 1)],
                                     hm1[:, 128 * c:128 * (c + 1)], idb32,
                                     start=True, stop=True)
                nc.vector.tensor_copy(hm1Ta[:], p1[:])
                p1b = psTR.tile([128, 4 * BD], F32, tag="tr")
                for c in range(4):
                    nc.tensor.matmul(p1b[:, BD * c:BD * (c + 1)],
                                     hm1[:, 512 + 128 * c:640 + 128 * c],
                                     idb32, start=True, stop=True)
                nc.vector.tensor_copy(hm1Tb[:], p1b[:])

                m2 = psB.tile([BD, M2], F32, tag="gB")
                for c in range(8):
                    s = (hm1Ta if c < 4 else hm1Tb)[:, BD * (c % 4):
                                                    BD * (c % 4 + 1)]
                    nc.tensor.matmul(m2[:], s, dm2_r[c][:],
                                     start=(c == 0), stop=False)
                nc.tensor.matmul(m2[:], ones_b[0:1, 0:BD],
                                 bias_r[0:1, 2048:2560], start=False, stop=True)
                hm2 = tq.tile([BD, M2], BF16, tag="hm2")
                nc.scalar.activation(hm2[:], m2[:], AF.Relu)
                p2 = psTR.tile([128, 4 * BD], F32, tag="tr")
                for c in range(4):
                    nc.tensor.matmul(p2[:, BD * c:BD * (c + 1)],
                                     hm2[:, 128 * c:128 * (c + 1)], idb32,
                                     start=True, stop=True)
                hm2T = tq.tile([128, 4 * BD], BF16, tag="hm2T")
                nc.vector.tensor_copy(hm2T[:], p2[:])

                yp_ps = psC.tile([BD, NY], F32, tag="gC")
                for c in range(4):
                    nc.tensor.matmul(yp_ps[:], hm2T[:, BD * c:BD * (c + 1)],
                                     dow_r[c][:], start=(c == 0), stop=False)
                nc.tensor.matmul(yp_ps[:], ones_b[0:1, 0:BD],
                                 bias_r[0:1, 2560:2624],
                                 start=False, stop=True)
                y_sb = tp.tile([BD, NY], F32, tag="y_sb")
                nc.scalar.copy(y_sb[:], yp_ps[:])
                nc.sync.dma_start(d_out[:, NY * t:NY * (t + 1)], y_sb[:])
                if t + 1 < hor:
                    y_b = tp.tile([BD, NY], BF16, tag="y_b")
                    nc.vector.tensor_copy(y_b[:], yp_ps[:])
                    p3 = psTR.tile([NX, BD], F32, tag="tr")
                    nc.tensor.matmul(p3[:], y_b[:], idb32,
                                     start=True, stop=True)
                    ypT_new = st.tile([96, BD], BF16, tag="ypT")
                    nc.vector.tensor_copy(ypT_new[0:NX, :], p3[:])
                    nc.vector.tensor_copy(ypT_new[64:96, :],
                                          id_hi_f[64:96, :])
                    ypT = ypT_new

    nc.compile()
    return nc


# ---------------------------------------------------------------------------
# Host-side sharding
# ---------------------------------------------------------------------------

def shard_inputs(inp, et=100, x_real=50, hor=60):
    f32 = np.float32
    x, y = np.asarray(inp["x"], f32), np.asarray(inp["y"], f32)
    tx = x.shape[1]
    chains = [("xf", False, x), ("xb", True, x),
              ("ef", False, y), ("eb", True, y)]
    in_maps = []
    shared = {}

    def bcast(a):
        return np.ascontiguousarray(a).astype(BF16NP)

    def wih_aug(pre):
        wih = np.asarray(inp[pre + "_Wih"], f32)
        bih = np.asarray(inp[pre + "_bih"], f32)
        bhh = np.asarray(inp[pre + "_bhh"], f32)
        aug = np.zeros((66, G), f32)
        aug[0:64, :] = wih.T
        bias = bih.copy()
        bias[0:2 * H] += bhh[0:2 * H]
        aug[64, :] = bias
        aug[65, H:2 * H] = BIG
        return bcast(aug)

    d_Wih = np.asarray(inp["d_Wih"], f32)
    d_bih = np.asarray(inp["d_bih"], f32)
    d_bhh = np.asarray(inp["d_bhh"], f32)
    dc_b = d_bih.copy()
    dc_b[0:2 * H] += d_bhh[0:2 * H]

    shared["em_w1t"] = bcast(np.asarray(inp["em_W1"], f32).T)
    shared["em_w2t"] = bcast(np.asarray(inp["em_W2"], f32).T)
    shared["eo_wt"] = bcast(np.asarray(inp["eo_W"], f32).T)
    shared["dc_wt"] = bcast(d_Wih[:, 0:2 * H].T)
    bias_b = np.concatenate([
        dc_b, np.asarray(inp["em_b1"], f32), np.asarray(inp["em_b2"], f32),
        np.asarray(inp["eo_b"], f32)])
    shared["bias_b"] = bcast(bias_b[None, :])
    shared["dwy_t"] = bcast(d_Wih[:, 2 * H:].T)
    shared["dwhh_t"] = bcast(np.asarray(inp["d_Whh"], f32).T)
    shared["dm_w1t"] = bcast(np.asarray(inp["dm_W1"], f32).T)
    shared["dm_w2t"] = bcast(np.asarray(inp["dm_W2"], f32).T)
    shared["do_wt"] = bcast(np.asarray(inp["do_W"], f32).T)
    bias_r_base = np.concatenate([
        np.zeros(H, f32), d_bhh[2 * H:], np.asarray(inp["dm_b1"], f32),
        np.asarray(inp["dm_b2"], f32), np.asarray(inp["do_b"], f32)])

    for j in range(NCORE):
        chain, half = j // 2, j % 2
        pre, rev, seq = chains[chain]
        T = seq.shape[1]
        s = seq[128 * half:128 * (half + 1)]          # [128, T, 64]
        xin = np.zeros((66, et, BE), f32)
        xin[64, :, :] = 1.0
        pad = et - T
        if pad:
            xin[65, 0:pad, :] = 1.0
        order = np.arange(T)[::-1] if rev else np.arange(T)
        xin[0:64, pad:, :] = s[:, order, :].transpose(2, 1, 0)
        m = dict(shared)
        m["xin"] = bcast(xin.reshape(66, et * BE))
        m["wih_aug"] = wih_aug(pre)
        m["whh_t"] = bcast(np.asarray(inp[pre + "_Whh"], f32).T)
        bias_r = bias_r_base.copy()
        bias_r[0:H] = np.asarray(inp[pre + "_bhh"], f32)[2 * H:]
        m["bias_r"] = bcast(bias_r[None, :])
        xl = np.concatenate([x[16 * j:16 * j + 16, -1, :],
                             x[128 + 16 * j:128 + 16 * j + 16, -1, :]])
        m["xlast_t"] = bcast(xl.T)
        in_maps.append(m)
    return in_maps


def unshard(results, hor=60):
    out = np.zeros((B, hor, NY), np.float32)
    for j in range(NCORE):
        o = results[j]["out"].reshape(BD, hor, NY)
        out[16 * j:16 * j + 16] = o[0:16]
        out[128 + 16 * j:128 + 16 * j + 16] = o[16:32]
    return out


_NC = None


def kernel(**inputs):
    global _NC
    from concourse.bass_utils import run_bass_kernel_spmd
    if _NC is None:
        _NC = build_nc()
    in_maps = shard_inputs(inputs)
    res = run_bass_kernel_spmd(_NC, in_maps, core_ids=list(range(NCORE)))
    return unshard(res.results)
